# revision 12
# baseline (speedup 1.0000x reference)
"""Trainium2 Bass kernel for the 4-layer Mamba-style GBM model.

Sharding: 8 cores = 4 batches x 2 d_inner halves. Each core handles one
batch and one 512-channel half of d_inner; the two cores of a batch pair
all-reduce the xproj output (dbl) and the out_proj partial sums.

Layout: activations are feature-major in SBUF ([d on partitions, t on
free]).  The selective scan runs as native tensor_tensor_scan calls
(one per d-group x state dim) with decays dA_n = exp(-n*sp) produced on
the scalar engine from sp = softplus(dt_raw) = Ln(Exp(dt_raw)+1) --
exp/ln only, so the whole dt/dA path stays on one activation table.
Work is spread across engines: scans on GPSIMD(Pool), dBu/C-multiplies
on DVE, the n-reduction as identity-matmul PSUM accumulation on PE.
"""
import sys
sys.path.insert(0, "/opt/trn_rl_repo")

import numpy as np
import ml_dtypes

import concourse.bacc as bacc
import concourse.tile as tile
from concourse import mybir
from concourse.bass_utils import run_bass_kernel_spmd

F32 = mybir.dt.float32
BF16 = mybir.dt.bfloat16
AF = mybir.ActivationFunctionType
OP = mybir.AluOpType
AX = mybir.AxisListType

D_MODEL = 512
D_LOC = 512          # d_inner half per core
N = 16               # d_state
S = 1024
KCONV = 4
NLAYERS = 4
LATENT = 1024
BATCH = 4
GROUPS = [[0, 1], [2, 3], [4, 5], [6, 7]]
NV = 96              # pvec columns

_CACHE = {}
NO_CC = False    # replace collectives with local copies (for TimelineSim)
# NOTE: tensor_tensor_scan is DVE-only on TRN2 (ISA check rejects Pool).
SCAN_POOL = [False] * 8
CMUL_POOL = [True] * 4    # C-multiply per g on Pool instead of DVE
DBN_POOL = [False] * 4    # dBu multiply per g on Pool instead of DVE


def _body(nc, tc, dram, out_d):
    import contextlib
    ctx = contextlib.ExitStack()
    with ctx:
        persist = ctx.enter_context(tc.tile_pool(name="persist", bufs=1))
        wbig = ctx.enter_context(tc.tile_pool(name="wbig", bufs=1))
        wsm = ctx.enter_context(tc.tile_pool(name="wsm", bufs=2))
        act = ctx.enter_context(tc.tile_pool(name="act", bufs=1))
        trans = ctx.enter_context(tc.tile_pool(name="trans", bufs=2))
        scanp = ctx.enter_context(tc.tile_pool(name="scanp", bufs=4))
        ps_mm = ctx.enter_context(tc.tile_pool(name="ps_mm", bufs=2, space="PSUM"))
        ps_sm = ctx.enter_context(tc.tile_pool(name="ps_sm", bufs=1, space="PSUM"))
        ps_y = ctx.enter_context(tc.tile_pool(name="ps_y", bufs=2, space="PSUM"))
        dpool = ctx.enter_context(tc.tile_pool(name="dpool", bufs=2, space="DRAM"))

        # ---- persistent small tensors
        pv = persist.tile([128, 4, NV], F32)
        nc.sync.dma_start(pv[:], dram["pvec"][:])
        l1b = persist.tile([128, 4], F32)
        nc.sync.dma_start(l1b[:], dram["lin1bT"][:])
        l2b = persist.tile([128, 8], F32)
        nc.sync.dma_start(l2b[:], dram["lin2bT"][:])
        ones_sb = persist.tile([128, 1], BF16)
        nc.sync.dma_start(ones_sb[:], dram["ones1"][:])
        ident_sb = persist.tile([128, 128], BF16)
        nc.sync.dma_start(ident_sb[:], dram["ident"][:])

        def pcol(g, c):
            return pv[:, g, c:c + 1]

        eps_t = persist.tile([1, 1], F32)
        nc.gpsimd.memset(eps_t[:], 1e-5)

        h = persist.tile([128, 4, S], F32)

        # ---- lin1: h = lin1w.T @ xT + b   (scoped pool, freed after)
        with tc.tile_pool(name="lin1p", bufs=1) as lp:
            xT_sb = lp.tile([128, 8, S], BF16)
            nc.sync.dma_start(xT_sb[:], dram["xT"][:])
            l1w = lp.tile([128, 8, 512], BF16)
            nc.sync.dma_start(l1w[:], dram["lin1w"][:])
            for m in range(4):
                for f in range(2):
                    ps = ps_mm.tile([128, 512], F32, tag="pmm")
                    for kc in range(8):
                        nc.tensor.matmul(
                            ps[:], l1w[:, kc, m * 128:(m + 1) * 128],
                            xT_sb[:, kc, f * 512:(f + 1) * 512],
                            start=(kc == 0), stop=(kc == 7))
                    nc.scalar.activation(h[:, m, f * 512:(f + 1) * 512],
                                         ps[:], AF.Identity,
                                         bias=l1b[:, m:m + 1])

        # ---- layers (big scan tiles in a scoped pool, freed before tail)
        with tc.tile_pool(name="bigp", bufs=1) as big, \
             tc.tile_pool(name="bigp2", bufs=2) as big2:
            for l in range(NLAYERS):
                inw_sb = wbig.tile([128, 4, 1024], BF16, tag="inw")
                nc.sync.dma_start(inw_sb[:], dram["inw"][l])
                outw_sb = wbig.tile([128, 4, 512], BF16, tag="outw")
                nc.sync.dma_start(outw_sb[:], dram["outw"][l])
                xprojw_sb = wsm.tile([128, 4, 64], BF16, tag="xprojw")
                nc.sync.dma_start(xprojw_sb[:], dram["xprojw"][l])
                dtw_sb = wsm.tile([32, 512], BF16, tag="dtw")
                nc.sync.dma_start(dtw_sb[:], dram["dtw"][l])

                # rmsnorm -> hn16 (t-halved; f=0 overlaps f=1 allreduce)
                sq = act.tile([128, 4, S], BF16, tag="sq")
                s_rep = act.tile([128, S], F32, tag="s_rep")
                hn16 = act.tile([128, 4, S], BF16, tag="hn16")
                for f in range(2):
                    o = f * 512
                    s_t = trans.tile([1, 512], F32, tag="s_t",
                                     name=f"s_t{f}")
                    for g in range(4):
                        nc.scalar.activation(sq[:, g, o:o + 512],
                                             h[:, g, o:o + 512], AF.Square)
                    pss = ps_sm.tile([1, 512], F32, tag="pnorm")
                    for kc in range(4):
                        nc.tensor.matmul(pss[:], ones_sb[:],
                                         sq[:, kc, o:o + 512],
                                         start=(kc == 0), stop=(kc == 3))
                    nc.scalar.activation(s_t[:], pss[:], AF.Ln,
                                         scale=1.0 / D_MODEL, bias=eps_t[:])
                    nc.scalar.activation(s_t[:], s_t[:], AF.Exp,
                                         scale=-0.5)
                    s_dram = dpool.tile([1, 512], F32, tag="s_dram")
                    nc.sync.dma_start(s_dram[:], s_t[:])
                    nc.sync.dma_start(
                        s_rep[:, o:o + 512],
                        s_dram[:].broadcast_to([128, 512]))
                    for g in range(4):
                        nc.vector.scalar_tensor_tensor(
                            hn16[:, g, o:o + 512], in0=h[:, g, o:o + 512],
                            scalar=pcol(g, l),
                            in1=s_rep[:, o:o + 512], op0=OP.mult,
                            op1=OP.mult)

                # in_proj -> xp_pad (pre-activation), sz16 = silu(z)
                xp_pad = act.tile([128, 4, S + 3], BF16, tag="xp_pad")
                nc.gpsimd.memset(xp_pad[:, :, 0:3], 0.0)
                sz16 = act.tile([128, 4, S], BF16, tag="sz16")
                for m in range(8):
                    for f in range(2):
                        ps = ps_mm.tile([128, 512], F32, tag="pmm")
                        for kc in range(4):
                            nc.tensor.matmul(
                                ps[:], inw_sb[:, kc, m * 128:(m + 1) * 128],
                                hn16[:, kc, f * 512:(f + 1) * 512],
                                start=(kc == 0), stop=(kc == 3))
                        if m < 4:
                            nc.scalar.activation(
                                xp_pad[:, m, 3 + f * 512: 3 + (f + 1) * 512],
                                ps[:], AF.Copy)
                        else:
                            nc.scalar.activation(
                                sz16[:, m - 4, f * 512:(f + 1) * 512],
                                ps[:], AF.Silu)

                # causal depthwise conv + bias + silu -> xpa16 (full-S)
                xpa16 = act.tile([128, 4, S], BF16, tag="xpa16")
                for g in range(4):
                    c0 = trans.tile([128, S], BF16, tag="conv",
                                    name=f"cv{g}")
                    nc.vector.tensor_scalar_mul(c0[:], xp_pad[:, g, 0:S],
                                                pcol(g, 16 + 4 * l + 0))
                    for k in range(1, KCONV):
                        c1 = trans.tile([128, S], BF16, tag="conv",
                                        name=f"cv{g}_{k}")
                        nc.vector.scalar_tensor_tensor(
                            c1[:], in0=xp_pad[:, g, k:k + S],
                            scalar=pcol(g, 16 + 4 * l + k),
                            in1=c0[:], op0=OP.mult, op1=OP.add)
                        c0 = c1
                    nc.scalar.activation(xpa16[:, g, :], c0[:],
                                         AF.Silu, bias=pcol(g, 8 + l))

                # xproj -> dbl partial -> pair allreduce in bf16
                # (B/C broadcasts read the collective output directly)
                dbl16 = trans.tile([64, S], BF16, tag="dbl16", bufs=1)
                dbl_outs = []
                dblp_full = trans.tile([64, S], BF16, tag="dblp", bufs=1)
                for fh in range(2):
                    o = fh * 512
                    psx = ps_sm.tile([64, 512], F32, tag="pxproj")
                    for kc in range(4):
                        nc.tensor.matmul(psx[:], xprojw_sb[:, kc, :],
                                         xpa16[:, kc, o:o + 512],
                                         start=(kc == 0), stop=(kc == 3))
                    nc.scalar.activation(dblp_full[:, o:o + 512], psx[:],
                                         AF.Copy)
                    dbl_in = dpool.tile([64, 512], BF16, tag="dbl_in")
                    dbl_out = dpool.tile([64, 512], BF16, tag="dbl_out")
                    nc.gpsimd.dma_start(dbl_in[:], dblp_full[:, o:o + 512])
                    if NO_CC:
                        nc.gpsimd.dma_start(dbl_out[:], dbl_in[:])
                    else:
                        nc.gpsimd.collective_compute(
                            "AllReduce", OP.add, replica_groups=GROUPS,
                            ins=[dbl_in[:].opt()],
                            outs=[dbl_out[:].opt()])
                    dbl_outs.append(dbl_out)
                    nc.gpsimd.dma_start(dbl16[:, o:o + 512], dbl_out[:])

                # dt-proj -> sp = softplus(dt_raw + dt_b) = Ln(Exp(.)+1)
                # (exp/ln only -- keeps the activation table fixed)
                sp16 = act.tile([128, 4, S], BF16, tag="xp_pad")
                for m in range(4):
                    for f in range(2):
                        ps = ps_mm.tile([128, 512], F32, tag="pmm")
                        nc.tensor.matmul(
                            ps[:], dtw_sb[:, m * 128:(m + 1) * 128],
                            dbl16[0:32, f * 512:(f + 1) * 512],
                            start=True, stop=True)
                        nc.scalar.activation(
                            sp16[:, m, f * 512:(f + 1) * 512], ps[:],
                            AF.Exp, bias=pcol(m, 4 + l))
                for g in range(4):
                    nc.scalar.activation(sp16[:, g, :], sp16[:, g, :],
                                         AF.Ln, bias=1.0)
                # dtu = dt * u = sp * xpa
                dtu16 = act.tile([128, 4, S], BF16, tag="hn16")
                for g in range(4):
                    nc.vector.tensor_tensor(dtu16[:, g, :], sp16[:, g, :],
                                            xpa16[:, g, :], OP.mult)

                # ---- selective scan, n in two halves of 8
                # per (nh, g): batched dBu on DVE, 8 scans (Pool), C-mult
                # (DVE), then PE accumulates the 8 n-slices into PSUM.
                y16 = act.tile([128, 4, S], BF16, tag="sq")
                ygp = act.tile([128, 4, S], BF16, tag="ypart")
                for nh in range(2):
                    B_rep = big.tile([128, 8, S], BF16, tag="B_rep")
                    C_rep = big.tile([128, 8, S], BF16, tag="C_rep")
                    for fh in range(2):
                        o = fh * 512
                        nc.sync.dma_start(
                            B_rep[:, :, o:o + 512],
                            dbl_outs[fh][32 + nh * 8:32 + nh * 8 + 8, :]
                            .unsqueeze(0).broadcast_to([128, 8, 512]))
                        nc.sync.dma_start(
                            C_rep[:, :, o:o + 512],
                            dbl_outs[fh][48 + nh * 8:48 + nh * 8 + 8, :]
                            .unsqueeze(0).broadcast_to([128, 8, 512]))
                    for g in range(4):
                        dBn = big2.tile([128, 8, S], BF16, tag="dBn")
                        dbn_eng = nc.gpsimd if DBN_POOL[g] else nc.vector
                        dbn_eng.tensor_tensor(
                            dBn[:],
                            dtu16[:, g, :].unsqueeze(1)
                            .broadcast_to([128, 8, S]),
                            B_rep[:], OP.mult)
                        hblk = big2.tile([128, 8, S], BF16, tag="hblk")
                        for j in range(8):
                            n = nh * 8 + j
                            dAn = scanp.tile([128, S], BF16, tag="dAn")
                            nc.scalar.activation(
                                dAn[:], sp16[:, g, :], AF.Exp,
                                scale=pcol(g, 32 + 16 * l + n))
                            sc_eng = nc.gpsimd if SCAN_POOL[j] else nc.vector
                            sc_eng.tensor_tensor_scan(
                                hblk[:, j, :], dAn[:], dBn[:, j, :], 0.0,
                                OP.mult, OP.add)
                        cm_eng = nc.gpsimd if CMUL_POOL[g] else nc.vector
                        cm_eng.tensor_tensor(hblk[:], hblk[:], C_rep[:],
                                             OP.mult)
                        # n-reduction on PE: psum += I @ hblk[:, j, :]
                        for f in range(2):
                            o = f * 512
                            psy = ps_y.tile([128, 512], F32, tag="psy")
                            for j in range(8):
                                nc.tensor.matmul(
                                    psy[:], ident_sb[:],
                                    hblk[:, j, o:o + 512],
                                    start=(j == 0), stop=(j == 7))
                            if nh == 0:
                                nc.scalar.activation(
                                    ygp[:, g, o:o + 512], psy[:], AF.Copy)
                            else:
                                yg = trans.tile([128, 512], BF16, tag="yg")
                                nc.vector.scalar_tensor_tensor(
                                    yg[:], in0=xpa16[:, g, o:o + 512],
                                    scalar=pcol(g, 12 + l),
                                    in1=psy[:], op0=OP.mult, op1=OP.add)
                                nc.vector.tensor_tensor(
                                    yg[:], yg[:], ygp[:, g, o:o + 512],
                                    OP.add)
                                nc.vector.tensor_tensor(
                                    y16[:, g, o:o + 512], yg[:],
                                    sz16[:, g, o:o + 512], OP.mult)

                # ---- out_proj partial + pair allreduce + residual add
                ypart = act.tile([128, 4, S], BF16, tag="ypart")
                for f in range(2):
                    o = f * 512
                    ysum = act.tile([128, 4, 512], BF16, tag="ysum",
                                    name=f"ysum{f}")
                    for m in range(4):
                        po = ps_mm.tile([128, 512], F32, tag="pmm")
                        for kc in range(4):
                            nc.tensor.matmul(
                                po[:],
                                outw_sb[:, kc, m * 128:(m + 1) * 128],
                                y16[:, kc, o:o + 512],
                                start=(kc == 0), stop=(kc == 3))
                        nc.scalar.activation(
                            ypart[:, m, o:o + 512], po[:], AF.Copy)
                    yp_in = dpool.tile([128, 4, 512], BF16, tag="yp_in")
                    yp_out = dpool.tile([128, 4, 512], BF16, tag="yp_out")
                    nc.gpsimd.dma_start(yp_in[:], ypart[:, :, o:o + 512])
                    if NO_CC:
                        nc.gpsimd.dma_start(yp_out[:], yp_in[:])
                    else:
                        nc.gpsimd.collective_compute(
                            "AllReduce", OP.add,
                            replica_groups=GROUPS,
                            ins=[yp_in[:].opt()],
                            outs=[yp_out[:].opt()])
                    nc.gpsimd.dma_start(ysum[:], yp_out[:])
                    for g in range(4):
                        nc.vector.tensor_tensor(
                            h[:, g, o:o + 512], h[:, g, o:o + 512],
                            ysum[:, g, :], OP.add)

        # ---- lin2 + transpose + softmax (all 1024 tokens; host slices)
        with tc.tile_pool(name="tailp", bufs=1) as tp, \
             tc.tile_pool(name="tailt", bufs=2) as tt:
            h16 = tp.tile([128, 4, S], BF16)
            for g in range(4):
                nc.vector.tensor_copy(h16[:, g, :], h[:, g, :])
            l2w = tp.tile([128, 4, 1024], BF16)
            nc.sync.dma_start(l2w[:], dram["lin2w"][:])
            lgt16 = tp.tile([128, 8, S], BF16)
            ps_tail = ctx.enter_context(
                tc.tile_pool(name="ps_tail", bufs=1, space="PSUM"))
            for f in range(2):
                for m in range(8):
                    ps = ps_mm.tile([128, 512], F32, tag="pmm")
                    for kc in range(4):
                        nc.tensor.matmul(
                            ps[:], l2w[:, kc, m * 128:(m + 1) * 128],
                            h16[:, kc, f * 512:(f + 1) * 512],
                            start=(kc == 0), stop=(kc == 3))
                    nc.scalar.activation(lgt16[:, m, f * 512:(f + 1) * 512],
                                         ps[:], AF.Identity,
                                         bias=l2b[:, m:m + 1])
            for tchunk in range(8):
                pst = ps_tail.tile([128, 1024], BF16, tag="ptr")
                for lc in range(8):
                    nc.tensor.transpose(
                        pst[:, lc * 128:(lc + 1) * 128],
                        lgt16[:, lc, tchunk * 128:(tchunk + 1) * 128],
                        ident_sb[:])
                eg = tt.tile([128, 1024], F32, tag="eg")
                nc.scalar.activation(eg[:], pst[:], AF.Exp)
                den = tt.tile([128, 32], F32, tag="den")
                nc.vector.tensor_reduce(
                    den[:], eg[:].rearrange("p (d c) -> p d c", c=32),
                    AX.X, OP.add)
                rec = tt.tile([128, 32], F32, tag="rec")
                nc.vector.reciprocal(rec[:], den[:])
                outt = tt.tile([128, 1024], F32, tag="outt")
                nc.vector.tensor_tensor(
                    outt[:].rearrange("p (d c) -> p d c", c=32),
                    eg[:].rearrange("p (d c) -> p d c", c=32),
                    rec[:].unsqueeze(2).broadcast_to([128, 32, 32]), OP.mult)
                nc.sync.dma_start(out_d[tchunk * 128:(tchunk + 1) * 128, :],
                                  outt[:])


def _build_nc():
    nc = bacc.Bacc("TRN2", target_bir_lowering=False, debug=False,
                   num_devices=8)
    dram = {}
    def din(name, shape, dt=BF16):
        dram[name] = nc.dram_tensor(name, shape, dt, kind="ExternalInput").ap()

    din("xT", [128, 8, S])
    din("lin1w", [128, 8, 512])
    din("lin2w", [128, 4, 1024])
    din("inw", [NLAYERS, 128, 4, 1024])
    din("outw", [NLAYERS, 128, 4, 512])
    din("xprojw", [NLAYERS, 128, 4, 64])
    din("dtw", [NLAYERS, 32, 512])
    din("pvec", [128, 4, NV], F32)
    din("lin1bT", [128, 4], F32)
    din("lin2bT", [128, 8], F32)
    din("ones1", [128, 1])
    din("ident", [128, 128])
    out_d = nc.dram_tensor("out_full", [S, LATENT], F32,
                           kind="ExternalOutput").ap()
    with tile.TileContext(nc) as tc:
        _body(nc, tc, dram, out_d)
    nc.compile()
    return nc


def _prep_inputs(x, lin1_w, lin1_b, norm_w, in_w, conv_w, conv_b, xproj_w,
                 dt_w, dt_b, A_log, Dp, out_w, lin2_w, lin2_b):
    bf = ml_dtypes.bfloat16
    f32 = np.float32
    x = np.asarray(x, f32)
    negA = np.exp(np.asarray(A_log, f32))                 # = n, (L, 1024, 16)
    in_w = np.asarray(in_w, f32)
    shared = {}
    shared["lin1w"] = np.ascontiguousarray(
        np.asarray(lin1_w, f32).reshape(8, 128, 512).transpose(1, 0, 2)
    ).astype(bf)
    shared["lin2w"] = np.ascontiguousarray(
        np.asarray(lin2_w, f32).reshape(4, 128, 1024).transpose(1, 0, 2)
    ).astype(bf)
    shared["lin1bT"] = np.ascontiguousarray(
        np.asarray(lin1_b, f32).reshape(4, 128).T)
    shared["lin2bT"] = np.ascontiguousarray(
        np.asarray(lin2_b, f32).reshape(8, 128).T)
    shared["ones1"] = np.ones((128, 1), bf)
    shared["ident"] = np.eye(128, dtype=f32).astype(bf)

    in_maps = []
    for c in range(8):
        b, half = c // 2, c % 2
        sl = slice(half * D_LOC, (half + 1) * D_LOC)
        m = dict(shared)
        m["xT"] = np.ascontiguousarray(
            x[b].T.reshape(8, 128, S).transpose(1, 0, 2)).astype(bf)
        inw = np.concatenate([in_w[:, :, sl],
                              in_w[:, :, 1024 + half * 512:
                                   1024 + (half + 1) * 512]], axis=2)
        m["inw"] = np.ascontiguousarray(
            inw.reshape(NLAYERS, 4, 128, 1024).transpose(0, 2, 1, 3)
        ).astype(bf)
        m["outw"] = np.ascontiguousarray(
            np.asarray(out_w, f32)[:, sl, :].reshape(NLAYERS, 4, 128, 512)
            .transpose(0, 2, 1, 3)).astype(bf)
        m["xprojw"] = np.ascontiguousarray(
            np.asarray(xproj_w, f32)[:, sl, :].reshape(NLAYERS, 4, 128, 64)
            .transpose(0, 2, 1, 3)).astype(bf)
        m["dtw"] = np.ascontiguousarray(
            np.asarray(dt_w, f32)[:, :, sl]).astype(bf)
        pvec = np.zeros((4, 128, NV), f32)
        for l in range(NLAYERS):
            pvec[:, :, l] = np.asarray(norm_w, f32)[l].reshape(4, 128)
            pvec[:, :, 4 + l] = np.asarray(dt_b, f32)[l, sl].reshape(4, 128)
            pvec[:, :, 8 + l] = np.asarray(conv_b, f32)[l, sl].reshape(4, 128)
            pvec[:, :, 12 + l] = np.asarray(Dp, f32)[l, sl].reshape(4, 128)
            for k in range(KCONV):
                pvec[:, :, 16 + 4 * l + k] = \
                    np.asarray(conv_w, f32)[l, sl, k].reshape(4, 128)
            for n in range(N):
                pvec[:, :, 32 + 16 * l + n] = -negA[l, sl, n].reshape(4, 128)
        m["pvec"] = np.ascontiguousarray(pvec.transpose(1, 0, 2))
        in_maps.append(m)
    return in_maps


def kernel(**inputs) -> np.ndarray:
    if "nc" not in _CACHE:
        _CACHE["nc"] = _build_nc()
    nc = _CACHE["nc"]
    in_maps = _prep_inputs(**inputs)
    res = run_bass_kernel_spmd(nc, in_maps, list(range(8)))
    out = np.zeros((BATCH, S, LATENT), np.float32)
    for b in range(BATCH):
        out[b, 0:512] = res.results[2 * b]["out_full"][0:512]
        out[b, 512:1024] = res.results[2 * b + 1]["out_full"][512:1024]
    return out


# revision 21
# speedup vs baseline: 1.0129x; 1.0129x over previous
"""Trainium2 Bass kernel for the 4-layer Mamba-style GBM model.

Sharding: 8 cores = 4 batches x 2 d_inner halves. Each core handles one
batch and one 512-channel half of d_inner; the two cores of a batch pair
all-reduce the xproj output (dbl) and the out_proj partial sums.

Layout: activations are feature-major in SBUF ([d on partitions, t on
free]).  The selective scan runs as native tensor_tensor_scan calls
(one per d-group x state dim) with decays dA_n = exp(-n*sp) produced on
the scalar engine from sp = softplus(dt_raw) = Ln(Exp(dt_raw)+1) --
exp/ln only, so the whole dt/dA path stays on one activation table.
Work is spread across engines: scans on GPSIMD(Pool), dBu/C-multiplies
on DVE, the n-reduction as identity-matmul PSUM accumulation on PE.
"""
import sys
sys.path.insert(0, "/opt/trn_rl_repo")

import numpy as np
import ml_dtypes

import concourse.bacc as bacc
import concourse.tile as tile
from concourse import mybir
from concourse.bass_utils import run_bass_kernel_spmd

F32 = mybir.dt.float32
BF16 = mybir.dt.bfloat16
AF = mybir.ActivationFunctionType
OP = mybir.AluOpType
AX = mybir.AxisListType

D_MODEL = 512
D_LOC = 512          # d_inner half per core
N = 16               # d_state
S = 1024
KCONV = 4
NLAYERS = 4
LATENT = 1024
BATCH = 4
GROUPS = [[0, 1], [2, 3], [4, 5], [6, 7]]
NV = 96              # pvec columns

_CACHE = {}
NO_CC = False    # replace collectives with local copies (for TimelineSim)
# NOTE: tensor_tensor_scan is DVE-only on TRN2 (ISA check rejects Pool).
# Pool(gpsimd) is ~2-3.8x slower per element than DVE, so it gets the
# small/cheap ops plus half the dBu multiplies; DVE keeps scans + C-mult.
CMUL_POOL = [False] * 4   # C-multiply per g on Pool instead of DVE
DBN_POOL = [True, False, True, False]   # dBu multiply per g on Pool
EVAC_POOL = False  # Pool cannot access PSUM (BIR verifier) -- keep False
HN_POOL = False    # STT illegal on Pool (ISA) -- keep False
DTU_POOL = False   # dtu on DVE (serial head; Pool too slow there)
GATE_POOL = True   # y*silu(z) TT on Pool
RESID_POOL = False # resid on DVE (serial head; Pool too slow there)


def _body(nc, tc, dram, out_d):
    import contextlib
    ctx = contextlib.ExitStack()
    with ctx:
        persist = ctx.enter_context(tc.tile_pool(name="persist", bufs=1))
        wbig = ctx.enter_context(tc.tile_pool(name="wbig", bufs=1))
        wsm = ctx.enter_context(tc.tile_pool(name="wsm", bufs=2))
        act = ctx.enter_context(tc.tile_pool(name="act", bufs=1))
        trans = ctx.enter_context(tc.tile_pool(name="trans", bufs=2))
        scanp = ctx.enter_context(tc.tile_pool(name="scanp", bufs=6))
        ps_mm = ctx.enter_context(tc.tile_pool(name="ps_mm", bufs=2, space="PSUM"))
        ps_sm = ctx.enter_context(tc.tile_pool(name="ps_sm", bufs=1, space="PSUM"))
        ps_y = ctx.enter_context(tc.tile_pool(name="ps_y", bufs=2, space="PSUM"))
        dpool = ctx.enter_context(tc.tile_pool(name="dpool", bufs=2, space="DRAM"))

        # ---- persistent small tensors
        pv = persist.tile([128, 4, NV], F32)
        nc.sync.dma_start(pv[:], dram["pvec"][:])
        l1b = persist.tile([128, 4], F32)
        nc.sync.dma_start(l1b[:], dram["lin1bT"][:])
        l2b = persist.tile([128, 8], F32)
        nc.sync.dma_start(l2b[:], dram["lin2bT"][:])
        ones_sb = persist.tile([128, 1], BF16)
        nc.sync.dma_start(ones_sb[:], dram["ones1"][:])
        ident_sb = persist.tile([128, 128], BF16)
        nc.sync.dma_start(ident_sb[:], dram["ident"][:])
        ones_row = persist.tile([1, 128], F32)
        nc.sync.dma_start(ones_row[:], dram["ones_row"][:])

        def pcol(g, c):
            return pv[:, g, c:c + 1]

        eps_t = persist.tile([1, 1], F32)
        nc.gpsimd.memset(eps_t[:], 1e-5)

        h = persist.tile([128, 4, S], F32)

        # ---- lin1: h = lin1w.T @ xT + b   (scoped pool, freed after)
        with tc.tile_pool(name="lin1p", bufs=1) as lp:
            xT_sb = lp.tile([128, 8, S], BF16)
            nc.sync.dma_start(xT_sb[:], dram["xT"][:])
            l1w = lp.tile([128, 8, 512], BF16)
            nc.sync.dma_start(l1w[:], dram["lin1w"][:])
            for m in range(4):
                for f in range(2):
                    ps = ps_mm.tile([128, 512], F32, tag="pmm")
                    for kc in range(8):
                        nc.tensor.matmul(
                            ps[:], l1w[:, kc, m * 128:(m + 1) * 128],
                            xT_sb[:, kc, f * 512:(f + 1) * 512],
                            start=(kc == 0), stop=(kc == 7))
                    nc.scalar.activation(h[:, m, f * 512:(f + 1) * 512],
                                         ps[:], AF.Identity,
                                         bias=l1b[:, m:m + 1])

        # ---- layers (big scan tiles in a scoped pool, freed before tail)
        with tc.tile_pool(name="bigp", bufs=1) as big, \
             tc.tile_pool(name="bigp2", bufs=2) as big2:
            for l in range(NLAYERS):
                inw_sb = wbig.tile([128, 4, 1024], BF16, tag="inw")
                nc.sync.dma_start(inw_sb[:], dram["inw"][l])
                outw_sb = wbig.tile([128, 4, 512], BF16, tag="outw")
                nc.sync.dma_start(outw_sb[:], dram["outw"][l])
                xprojw_sb = wsm.tile([128, 4, 64], BF16, tag="xprojw")
                nc.sync.dma_start(xprojw_sb[:], dram["xprojw"][l])
                dtw_sb = wsm.tile([32, 512], BF16, tag="dtw")
                nc.sync.dma_start(dtw_sb[:], dram["dtw"][l])

                # rmsnorm -> hn16 (t-halved; f=0 overlaps f=1 allreduce)
                sq = act.tile([128, 4, S], BF16, tag="sq")
                hn16 = act.tile([128, 4, S], BF16, tag="hn16")
                for f in range(2):
                    o = f * 512
                    s_t = trans.tile([1, 512], F32, tag="s_t",
                                     name=f"s_t{f}")
                    for g in range(4):
                        nc.scalar.activation(sq[:, g, o:o + 512],
                                             h[:, g, o:o + 512], AF.Square)
                    pss = ps_sm.tile([1, 512], F32, tag="pnorm")
                    for kc in range(4):
                        nc.tensor.matmul(pss[:], ones_sb[:],
                                         sq[:, kc, o:o + 512],
                                         start=(kc == 0), stop=(kc == 3))
                    # s = 1/sqrt(mean + eps): Sqrt on Act (shares the
                    # square/copy table), reciprocal on DVE
                    nc.scalar.activation(s_t[:], pss[:], AF.Sqrt,
                                         scale=1.0 / D_MODEL, bias=eps_t[:])
                    nc.vector.reciprocal(s_t[:], s_t[:])
                    # broadcast s to all partitions via K=1 matmul
                    s_rep = ps_sm.tile([128, 512], F32, tag="srep")
                    nc.tensor.matmul(s_rep[:], ones_row[:], s_t[:],
                                     start=True, stop=True)
                    for g in range(4):
                        nc.vector.scalar_tensor_tensor(
                            hn16[:, g, o:o + 512], in0=h[:, g, o:o + 512],
                            scalar=pcol(g, l),
                            in1=s_rep[:], op0=OP.mult,
                            op1=OP.mult)

                # in_proj -> xp_pad (pre-activation), sz16 = silu(z)
                xp_pad = act.tile([128, 4, S + 3], BF16, tag="xp_pad")
                nc.gpsimd.memset(xp_pad[:, :, 0:3], 0.0)
                sz16 = act.tile([128, 4, S], BF16, tag="sz16")
                for m in range(8):
                    for f in range(2):
                        ps = ps_mm.tile([128, 512], F32, tag="pmm")
                        for kc in range(4):
                            nc.tensor.matmul(
                                ps[:], inw_sb[:, kc, m * 128:(m + 1) * 128],
                                hn16[:, kc, f * 512:(f + 1) * 512],
                                start=(kc == 0), stop=(kc == 3))
                        if m < 4:
                            if EVAC_POOL:
                                nc.gpsimd.tensor_copy(
                                    xp_pad[:, m,
                                           3 + f * 512: 3 + (f + 1) * 512],
                                    ps[:])
                            else:
                                nc.scalar.activation(
                                    xp_pad[:, m,
                                           3 + f * 512: 3 + (f + 1) * 512],
                                    ps[:], AF.Copy)
                        else:
                            nc.scalar.activation(
                                sz16[:, m - 4, f * 512:(f + 1) * 512],
                                ps[:], AF.Silu)

                # causal depthwise conv + bias + silu -> xpa16
                # (f-split so xproj f0 + its allreduce launch early)
                xpa16 = act.tile([128, 4, S], BF16, tag="xpa16")
                dbl16 = trans.tile([64, S], BF16, tag="dbl16", bufs=1)
                dblp_full = trans.tile([64, S], BF16, tag="dblp", bufs=1)
                dbl_outs = [None, None]
                for f in range(2):
                    o = f * 512
                    for g in range(4):
                        c0 = trans.tile([128, 512], BF16, tag="conv",
                                        name=f"cv{f}_{g}")
                        nc.vector.tensor_scalar_mul(
                            c0[:], xp_pad[:, g, o:o + 512],
                            pcol(g, 16 + 4 * l + 0))
                        for k in range(1, KCONV):
                            c1 = trans.tile([128, 512], BF16, tag="conv",
                                            name=f"cv{f}_{g}_{k}")
                            nc.vector.scalar_tensor_tensor(
                                c1[:], in0=xp_pad[:, g, o + k:o + k + 512],
                                scalar=pcol(g, 16 + 4 * l + k),
                                in1=c0[:], op0=OP.mult, op1=OP.add)
                            c0 = c1
                        nc.scalar.activation(xpa16[:, g, o:o + 512], c0[:],
                                             AF.Silu, bias=pcol(g, 8 + l))
                    # xproj f-half -> dbl partial -> pair allreduce
                    psx = ps_sm.tile([64, 512], F32, tag="pxproj")
                    for kc in range(4):
                        nc.tensor.matmul(psx[:], xprojw_sb[:, kc, :],
                                         xpa16[:, kc, o:o + 512],
                                         start=(kc == 0), stop=(kc == 3))
                    nc.scalar.activation(dblp_full[:, o:o + 512], psx[:],
                                         AF.Copy)
                    dbl_in = dpool.tile([64, 512], BF16, tag="dbl_in")
                    dbl_out = dpool.tile([64, 512], BF16, tag="dbl_out")
                    nc.gpsimd.dma_start(dbl_in[:], dblp_full[:, o:o + 512])
                    if NO_CC:
                        nc.gpsimd.dma_start(dbl_out[:], dbl_in[:])
                    else:
                        nc.gpsimd.collective_compute(
                            "AllReduce", OP.add, replica_groups=GROUPS,
                            ins=[dbl_in[:].opt()],
                            outs=[dbl_out[:].opt()])
                    dbl_outs[f] = dbl_out
                    nc.gpsimd.dma_start(dbl16[:, o:o + 512], dbl_out[:])

                # per f-half: dt -> sp -> dtu -> scans (chained via the
                # f0 end state) -> C-mult -> PE n-reduction -> gate ->
                # out_proj + allreduce + residual.  f1's scan phase
                # overlaps f0's out_proj/collective and the next layer's
                # f0 head overlaps f1's scans.
                sp16 = act.tile([128, 4, S], BF16, tag="xp_pad")
                dtu16 = act.tile([128, 4, S], BF16, tag="dtu")
                y16 = act.tile([128, 4, S], BF16, tag="sq")
                st_all = act.tile([128, 8, 8], BF16, tag="st")
                ypart = act.tile([128, 4, S], BF16, tag="ypart")
                for f in range(2):
                    o = f * 512
                    for m in range(4):
                        ps = ps_mm.tile([128, 512], F32, tag="pmm")
                        nc.tensor.matmul(
                            ps[:], dtw_sb[:, m * 128:(m + 1) * 128],
                            dbl16[0:32, o:o + 512],
                            start=True, stop=True)
                        nc.scalar.activation(
                            sp16[:, m, o:o + 512], ps[:],
                            AF.Exp, bias=pcol(m, 4 + l))
                    for g in range(4):
                        nc.scalar.activation(sp16[:, g, o:o + 512],
                                             sp16[:, g, o:o + 512],
                                             AF.Ln, bias=1.0)
                        dtu_eng = nc.gpsimd if DTU_POOL else nc.vector
                        dtu_eng.tensor_tensor(dtu16[:, g, o:o + 512],
                                              sp16[:, g, o:o + 512],
                                              xpa16[:, g, o:o + 512],
                                              OP.mult)
                    ygp = act.tile([128, 4, 512], BF16, tag="ygp",
                                   name=f"ygp{f}", bufs=2)
                    for nh in range(2):
                        B_rep = big.tile([128, 8, 512], BF16, tag="B_rep",
                                         bufs=2)
                        C_rep = big.tile([128, 8, 512], BF16, tag="C_rep",
                                         bufs=2)
                        nc.sync.dma_start(
                            B_rep[:],
                            dbl_outs[f][32 + nh * 8:32 + nh * 8 + 8, :]
                            .unsqueeze(0).broadcast_to([128, 8, 512]))
                        nc.sync.dma_start(
                            C_rep[:],
                            dbl_outs[f][48 + nh * 8:48 + nh * 8 + 8, :]
                            .unsqueeze(0).broadcast_to([128, 8, 512]))
                        for g in range(4):
                            u = nh * 4 + g
                            dBn = big2.tile([128, 8, 512], BF16, tag="dBn")
                            dbn_eng = nc.gpsimd if DBN_POOL[g] else nc.vector
                            for jh in range(2):
                                dbn_eng.tensor_tensor(
                                    dBn[:, jh * 4:jh * 4 + 4, :],
                                    dtu16[:, g, o:o + 512].unsqueeze(1)
                                    .broadcast_to([128, 4, 512]),
                                    B_rep[:, jh * 4:jh * 4 + 4, :], OP.mult)
                            hblk = big2.tile([128, 8, 512], BF16,
                                             tag="hblk")
                            for j in range(8):
                                n = nh * 8 + j
                                dAn = scanp.tile([128, 512], BF16,
                                                 tag="dAn")
                                nc.scalar.activation(
                                    dAn[:], sp16[:, g, o:o + 512], AF.Exp,
                                    scale=pcol(g, 32 + 16 * l + n))
                                init = (0.0 if f == 0
                                        else st_all[:, u, j:j + 1])
                                nc.vector.tensor_tensor_scan(
                                    hblk[:, j, :], dAn[:], dBn[:, j, :],
                                    init, OP.mult, OP.add)
                            if f == 0:
                                nc.vector.tensor_copy(
                                    st_all[:, u, :],
                                    hblk[:, :, 511:512]
                                    .rearrange("p a b -> p (a b)"))
                            cm_eng = nc.gpsimd if CMUL_POOL[g] else nc.vector
                            cm_eng.tensor_tensor(hblk[:], hblk[:],
                                                 C_rep[:], OP.mult)
                            # n-reduction on PE: psum += I @ hblk[:, j, :]
                            psy = ps_y.tile([128, 512], F32, tag="psy")
                            for j in range(8):
                                nc.tensor.matmul(
                                    psy[:], ident_sb[:], hblk[:, j, :],
                                    start=(j == 0), stop=(j == 7))
                            if nh == 0:
                                nc.scalar.activation(
                                    ygp[:, g, :], psy[:], AF.Copy)
                            else:
                                yg = trans.tile([128, 512], BF16, tag="yg")
                                nc.vector.scalar_tensor_tensor(
                                    yg[:], in0=xpa16[:, g, o:o + 512],
                                    scalar=pcol(g, 12 + l),
                                    in1=psy[:], op0=OP.mult, op1=OP.add)
                                nc.vector.tensor_tensor(
                                    yg[:], yg[:], ygp[:, g, :], OP.add)
                                gt_eng = (nc.gpsimd if GATE_POOL
                                          else nc.vector)
                                gt_eng.tensor_tensor(
                                    y16[:, g, o:o + 512], yg[:],
                                    sz16[:, g, o:o + 512], OP.mult)

                    # out_proj partial (f-half) + pair allreduce + resid
                    ysum = act.tile([128, 4, 512], BF16, tag="ysum",
                                    name=f"ysum{f}", bufs=2)
                    for m in range(4):
                        po = ps_mm.tile([128, 512], F32, tag="pmm")
                        for kc in range(4):
                            nc.tensor.matmul(
                                po[:],
                                outw_sb[:, kc, m * 128:(m + 1) * 128],
                                y16[:, kc, o:o + 512],
                                start=(kc == 0), stop=(kc == 3))
                        nc.scalar.activation(
                            ypart[:, m, o:o + 512], po[:], AF.Copy)
                    yp_in = dpool.tile([128, 4, 512], BF16, tag="yp_in")
                    yp_out = dpool.tile([128, 4, 512], BF16, tag="yp_out")
                    nc.gpsimd.dma_start(yp_in[:], ypart[:, :, o:o + 512])
                    if NO_CC:
                        nc.gpsimd.dma_start(yp_out[:], yp_in[:])
                    else:
                        nc.gpsimd.collective_compute(
                            "AllReduce", OP.add,
                            replica_groups=GROUPS,
                            ins=[yp_in[:].opt()],
                            outs=[yp_out[:].opt()])
                    nc.gpsimd.dma_start(ysum[:], yp_out[:])
                    rs_eng = nc.gpsimd if RESID_POOL else nc.vector
                    for g in range(4):
                        rs_eng.tensor_tensor(
                            h[:, g, o:o + 512], h[:, g, o:o + 512],
                            ysum[:, g, :], OP.add)

        # ---- lin2 + transpose + softmax (all 1024 tokens; host slices)
        with tc.tile_pool(name="tailp", bufs=1) as tp, \
             tc.tile_pool(name="tailt", bufs=2) as tt:
            h16 = tp.tile([128, 4, S], BF16)
            for g in range(4):
                nc.scalar.activation(h16[:, g, :], h[:, g, :], AF.Copy)
            l2w = tp.tile([128, 4, 1024], BF16)
            nc.sync.dma_start(l2w[:], dram["lin2w"][:])
            lgt16 = tp.tile([128, 8, S], BF16)
            ps_tail = ctx.enter_context(
                tc.tile_pool(name="ps_tail", bufs=1, space="PSUM"))
            for f in range(2):
                for m in range(8):
                    ps = ps_mm.tile([128, 512], F32, tag="pmm")
                    for kc in range(4):
                        nc.tensor.matmul(
                            ps[:], l2w[:, kc, m * 128:(m + 1) * 128],
                            h16[:, kc, f * 512:(f + 1) * 512],
                            start=(kc == 0), stop=(kc == 3))
                    nc.scalar.activation(lgt16[:, m, f * 512:(f + 1) * 512],
                                         ps[:], AF.Identity,
                                         bias=l2b[:, m:m + 1])
            for tchunk in range(8):
                pst = ps_tail.tile([128, 1024], BF16, tag="ptr")
                for lc in range(8):
                    nc.tensor.transpose(
                        pst[:, lc * 128:(lc + 1) * 128],
                        lgt16[:, lc, tchunk * 128:(tchunk + 1) * 128],
                        ident_sb[:])
                eg = tt.tile([128, 1024], F32, tag="eg")
                nc.scalar.activation(eg[:], pst[:], AF.Exp)
                den = tt.tile([128, 32], F32, tag="den")
                nc.vector.tensor_reduce(
                    den[:], eg[:].rearrange("p (d c) -> p d c", c=32),
                    AX.X, OP.add)
                rec = tt.tile([128, 32], F32, tag="rec")
                nc.vector.reciprocal(rec[:], den[:])
                outt = tt.tile([128, 1024], F32, tag="outt")
                nc.vector.tensor_tensor(
                    outt[:].rearrange("p (d c) -> p d c", c=32),
                    eg[:].rearrange("p (d c) -> p d c", c=32),
                    rec[:].unsqueeze(2).broadcast_to([128, 32, 32]), OP.mult)
                nc.sync.dma_start(out_d[tchunk * 128:(tchunk + 1) * 128, :],
                                  outt[:])


def _build_nc():
    nc = bacc.Bacc("TRN2", target_bir_lowering=False, debug=False,
                   num_devices=8)
    dram = {}
    def din(name, shape, dt=BF16):
        dram[name] = nc.dram_tensor(name, shape, dt, kind="ExternalInput").ap()

    din("xT", [128, 8, S])
    din("lin1w", [128, 8, 512])
    din("lin2w", [128, 4, 1024])
    din("inw", [NLAYERS, 128, 4, 1024])
    din("outw", [NLAYERS, 128, 4, 512])
    din("xprojw", [NLAYERS, 128, 4, 64])
    din("dtw", [NLAYERS, 32, 512])
    din("pvec", [128, 4, NV], F32)
    din("lin1bT", [128, 4], F32)
    din("lin2bT", [128, 8], F32)
    din("ones1", [128, 1])
    din("ones_row", [1, 128], F32)
    din("ident", [128, 128])
    out_d = nc.dram_tensor("out_full", [S, LATENT], F32,
                           kind="ExternalOutput").ap()
    with tile.TileContext(nc) as tc:
        _body(nc, tc, dram, out_d)
    nc.compile()
    return nc


def _prep_inputs(x, lin1_w, lin1_b, norm_w, in_w, conv_w, conv_b, xproj_w,
                 dt_w, dt_b, A_log, Dp, out_w, lin2_w, lin2_b):
    bf = ml_dtypes.bfloat16
    f32 = np.float32
    x = np.asarray(x, f32)
    negA = np.exp(np.asarray(A_log, f32))                 # = n, (L, 1024, 16)
    in_w = np.asarray(in_w, f32)
    shared = {}
    shared["lin1w"] = np.ascontiguousarray(
        np.asarray(lin1_w, f32).reshape(8, 128, 512).transpose(1, 0, 2)
    ).astype(bf)
    shared["lin2w"] = np.ascontiguousarray(
        np.asarray(lin2_w, f32).reshape(4, 128, 1024).transpose(1, 0, 2)
    ).astype(bf)
    shared["lin1bT"] = np.ascontiguousarray(
        np.asarray(lin1_b, f32).reshape(4, 128).T)
    shared["lin2bT"] = np.ascontiguousarray(
        np.asarray(lin2_b, f32).reshape(8, 128).T)
    shared["ones1"] = np.ones((128, 1), bf)
    shared["ones_row"] = np.ones((1, 128), f32)
    shared["ident"] = np.eye(128, dtype=f32).astype(bf)

    in_maps = []
    for c in range(8):
        b, half = c // 2, c % 2
        sl = slice(half * D_LOC, (half + 1) * D_LOC)
        m = dict(shared)
        m["xT"] = np.ascontiguousarray(
            x[b].T.reshape(8, 128, S).transpose(1, 0, 2)).astype(bf)
        inw = np.concatenate([in_w[:, :, sl],
                              in_w[:, :, 1024 + half * 512:
                                   1024 + (half + 1) * 512]], axis=2)
        m["inw"] = np.ascontiguousarray(
            inw.reshape(NLAYERS, 4, 128, 1024).transpose(0, 2, 1, 3)
        ).astype(bf)
        m["outw"] = np.ascontiguousarray(
            np.asarray(out_w, f32)[:, sl, :].reshape(NLAYERS, 4, 128, 512)
            .transpose(0, 2, 1, 3)).astype(bf)
        m["xprojw"] = np.ascontiguousarray(
            np.asarray(xproj_w, f32)[:, sl, :].reshape(NLAYERS, 4, 128, 64)
            .transpose(0, 2, 1, 3)).astype(bf)
        m["dtw"] = np.ascontiguousarray(
            np.asarray(dt_w, f32)[:, :, sl]).astype(bf)
        pvec = np.zeros((4, 128, NV), f32)
        for l in range(NLAYERS):
            pvec[:, :, l] = np.asarray(norm_w, f32)[l].reshape(4, 128)
            pvec[:, :, 4 + l] = np.asarray(dt_b, f32)[l, sl].reshape(4, 128)
            pvec[:, :, 8 + l] = np.asarray(conv_b, f32)[l, sl].reshape(4, 128)
            pvec[:, :, 12 + l] = np.asarray(Dp, f32)[l, sl].reshape(4, 128)
            for k in range(KCONV):
                pvec[:, :, 16 + 4 * l + k] = \
                    np.asarray(conv_w, f32)[l, sl, k].reshape(4, 128)
            for n in range(N):
                pvec[:, :, 32 + 16 * l + n] = -negA[l, sl, n].reshape(4, 128)
        m["pvec"] = np.ascontiguousarray(pvec.transpose(1, 0, 2))
        in_maps.append(m)
    return in_maps


def kernel(**inputs) -> np.ndarray:
    if "nc" not in _CACHE:
        _CACHE["nc"] = _build_nc()
    nc = _CACHE["nc"]
    in_maps = _prep_inputs(**inputs)
    res = run_bass_kernel_spmd(nc, in_maps, list(range(8)))
    out = np.zeros((BATCH, S, LATENT), np.float32)
    for b in range(BATCH):
        out[b, 0:512] = res.results[2 * b]["out_full"][0:512]
        out[b, 512:1024] = res.results[2 * b + 1]["out_full"][512:1024]
    return out
